# revision 5
# baseline (speedup 1.0000x reference)
"""AngProtoLoss (stable) distributed Bass kernel for 8 TRN2 NeuronCores.

Column-block scheme, NO device collectives (~59-64us vs 185us baseline):
  - Each core owns 512 speakers (columns k of the NxN cos matrix) and
    computes e[k, i] = exp(w*cos_ik) for ALL 4096 rows i.  The full u
    matrix (last utterance of every speaker) is shipped to every core from
    the host, pre-transposed to [d, i], pre-scaled by 4w/|u_i| per column,
    fp8 (host marshaling, like the baseline's shard slicing).  Columns are
    rotated by 512*c per core so the diagonal block always lands at
    i' = k_local: one SPMD program works for all cores.  x ships as fp8.
  - Per chunk of 128 speakers, everything heavy lives on the PE:
      * centroid sum: 8 DoubleRow matmuls against a static 0.25-identity
        (SWI layout) accumulate 0.25*sum_m x[k,m,:] in PSUM;
      * transpose: 4 matmuls against the identity -> cT fp8 (stationary);
      * cos: fp8 x fp8 DoubleRow matmuls, uT moving, 512 cols each -- the
        216ns/matmul fp8 roofline;
      * row sums: ones-vector matmuls reduce over the chunk's 128
        k-partitions, accumulated across chunks in 2 PSUM banks (explicit
        tile_position packs 4 one-row sums per bank).
    ACT does exp(S * rs_k) straight from PSUM pairs [128,1024] with the
    per-partition scale AP rs_k = 1/(4|csum_k|) (rsqrt via Ln+Exp, single
    pinned act table - InstLoadActFuncSet up front, zero switches).  DVE
    does the fused clip+bf16 epilogue: max(e,1) == exp(w*max(cos,eps)) up
    to 1e-5 rel, plus diag extraction via identity mask.
  - Outputs per core: 4096 partial exp-sums + 512 diagonal e_kk values.
    Host: s_i = sum over cores (after unrotating), cos_ii = log(e_ii)/w,
    loss = mean(log s_i - w*max(cos_ii, eps)).  (b cancels exactly.)
"""

import os
import sys

for _p in ("/opt/trn_rl_repo",):
    if os.path.isdir(_p) and _p not in sys.path:
        sys.path.append(_p)

import math

import numpy as np
import ml_dtypes

import concourse.bass as bass
import concourse.tile as tile
from concourse import bacc, mybir
from concourse.bass_utils import run_bass_kernel_spmd
from concourse.masks import make_identity

N_CORES = 8
N, M, D = 4096, 16, 512
P = 128
LOCAL = N // N_CORES        # 512 speaker columns per core
NCHUNK = LOCAL // P         # 4 chunks of 128 columns
NT = D // P                 # 4 d-subtiles of 128
NB = N // 512               # 8 i-bites of 512 columns of the moving tensor
EPS = 1e-6
UT_BOOST = 4.0              # folded into host uT scaling
CT_BOOST = 4.0              # folded into the rsqrt bias (ln 4)
EXP_SCALE = 1.0 / (UT_BOOST * CT_BOOST)

F32 = mybir.dt.float32
BF16 = mybir.dt.bfloat16
FP8 = mybir.dt.float8e4
AF = mybir.ActivationFunctionType
DR = mybir.MatmulPerfMode.DoubleRow


def build_program():
    nc = bacc.Bacc("TRN2", target_bir_lowering=False, debug=False,
                   num_devices=N_CORES)
    x = nc.dram_tensor("x", [LOCAL, M, D], FP8, kind="ExternalInput").ap()
    ut = nc.dram_tensor("ut", [D, N], FP8, kind="ExternalInput").ap()
    out = nc.dram_tensor("out", [9, 512], F32, kind="ExternalOutput").ap()

    with tile.TileContext(nc) as tc:
        _pin_act_table(nc)
        _build(nc, tc, x, ut, out)
    nc.compile()
    return nc


def _pin_act_table(nc):
    """Load the ln+exp table once up front so the compile-time table pass
    never has to thrash between natural_log and exp_and_others (each load
    is a ~1.3us TDRAM DMA)."""
    from concourse.hw_specs import get_activation_tables
    tables = list(get_activation_tables(nc.m.arch).keys())
    tid = tables.index("natural_log_exp_and_others")
    nc.scalar.add_instruction(mybir.InstLoadActFuncSet(
        name=nc.get_next_instruction_name(), ins=[], outs=[],
        act_func_set_id=tid))


def _build(nc, tc, x, ut, out):
    from contextlib import ExitStack
    ctx = ExitStack()
    with ctx:
        singles = ctx.enter_context(tc.tile_pool(name="singles", bufs=1))
        xpool = ctx.enter_context(tc.tile_pool(name="xpool", bufs=4))
        cpool = ctx.enter_context(tc.tile_pool(name="cpool", bufs=2))
        ctpool = ctx.enter_context(tc.tile_pool(name="ctpool", bufs=2))
        stats = ctx.enter_context(tc.tile_pool(name="stats", bufs=4))
        epool = ctx.enter_context(tc.tile_pool(name="epool", bufs=8))
        empool = ctx.enter_context(tc.tile_pool(name="empool", bufs=5))
        wpsum = ctx.enter_context(tc.tile_pool(name="wpsum", bufs=1, space="PSUM"))
        mpsum = ctx.enter_context(tc.tile_pool(name="mpsum", bufs=2, space="PSUM"))
        spsum = ctx.enter_context(tc.tile_pool(name="spsum", bufs=1, space="PSUM"))

        # identity (bf16): rhs of the transpose matmuls + diag-extract mask
        ident = singles.tile([P, P], BF16)
        make_identity(nc, ident)
        # 0.25-identity, fp8, in DoubleRowSwInterleave weight layout:
        # free position f = 2*(127-j)+s holds the (ksub=s, col j) weight,
        # i.e. nonzero iff 2*k + f - 254 - s == 0.  out[j, f] =
        # 0.25*(x[j,2m] + x[j,2m+1]) accumulated over m in PSUM.
        identq2 = singles.tile([P, 2, P], FP8)
        nc.gpsimd.memset(identq2, 0.0)
        iq2v = identq2.rearrange("p a b -> p (a b)")
        for s in range(2):
            nc.gpsimd.affine_select(
                out=iq2v, in_=iq2v,
                compare_op=mybir.AluOpType.not_equal,
                fill=0.25, base=-254 - s, pattern=[[1, 2 * P]],
                channel_multiplier=2)
        ones = singles.tile([P, 1], BF16)
        nc.vector.memset(ones, 1.0)
        e_diag = singles.tile([P, NCHUNK], F32)

        ut_sb = singles.tile([P, NT, N], FP8)       # [d%128, d//128, i]
        # persistent per-i partial sums: slot for bite b lives in tile b//4
        # at partition base 32*(b%4) (explicit tile_position allows base 96)
        sum_ps = [spsum.tile([P, 512], F32, name=f"sps{j}") for j in range(2)]

        # ---------- loads, all on the sync ring (the only DGE ring that
        # spreads across all 16 DMA queues): x0, then uT (needed by the
        # first main matmul, ~when tree(0)+transpose(0) finish), then the
        # remaining chunks ----------
        xs = []
        for r in range(NCHUNK):
            xr = xpool.tile([P, M, D], FP8, name=f"x{r}", tag="x")
            # two m-halves per chunk so the first tree matmuls can start
            # while the second half is still in flight
            nc.sync.dma_start(out=xr[:, :M // 2, :],
                              in_=x[r * P:(r + 1) * P, :M // 2, :])
            nc.sync.dma_start(out=xr[:, M // 2:, :],
                              in_=x[r * P:(r + 1) * P, M // 2:, :])
            xs.append(xr)
            # uT interleaves with the x stream in d-halves: the h0 matmuls
            # of chunk 0 need only t=0,1, so x1/x2/x3 land ~3us earlier
            # than with a single 2MiB uT transfer in the stream
            if r <= 1:
                utv = ut.rearrange("(t p) i -> p t i", p=P)
                nc.sync.dma_start(out=ut_sb[:, 2 * r:2 * r + 2, :],
                                  in_=utv[:, 2 * r:2 * r + 2, :])

        # Per-chunk work, software-pipelined: the epilogue of chunk r-1
        # (emax / diag extraction on DVE, partition-sum matmuls on PE) is
        # emitted during iteration r so it never head-of-line blocks the
        # next chunk's tree / transpose / main matmuls in the engine FIFOs.
        e_tiles = [None] * NCHUNK    # per chunk: [e pair tiles]
        em_tiles = [None] * NCHUNK   # per chunk: [(pair, emax tile)]
        rs_tiles = [None] * NCHUNK   # per chunk: 1/|csum| scale AP



        def epilogue(r):
            em_tiles[r] = []
            for pj, e in enumerate(e_tiles[r]):
                if pj == 0:
                    dscr = stats.tile([P, P], BF16, name=f"dg{r}", tag="dg")
                    nc.vector.tensor_mul(
                        dscr, e[:, r * P:(r + 1) * P], ident)
                    nc.vector.tensor_reduce(
                        e_diag[:, r:r + 1], dscr,
                        axis=mybir.AxisListType.X, op=mybir.AluOpType.add)
                em = empool.tile([P, 2 * 512], BF16, name=f"em{r}_{pj}",
                                 tag="em")
                nc.vector.tensor_scalar_max(em, e, 1.0)
                em_tiles[r].append((pj, em))

        for r in range(NCHUNK):
            xr = xs[r]
            # ---- centroid sum on the PE: 8 DoubleRow matmuls against the
            # static 0.25-identity, accumulating 0.25*sum_m x in PSUM ----
            cps = wpsum.tile([P, D], F32, name=f"cps{r}", tag="cps")
            for m2 in range(M // 2):
                nc.tensor.matmul(cps, identq2, xr[:, 2 * m2:2 * m2 + 2, :],
                                 start=(m2 == 0), stop=(m2 == M // 2 - 1),
                                 perf_mode=mybir.MatmulPerfMode.DoubleRowSwInterleave)
            csum = cpool.tile([P, D], BF16, name=f"csum{r}", tag="csum")
            nc.vector.tensor_copy(csum, cps)

            # previous chunk's DVE epilogue
            if r >= 1:
                epilogue(r - 1)

            # ---- rs = 1/(4*|csum|) = exp(-0.5*ln(16*ssq)); the 4 cancels
            # the host-side 4w/|u| boost times the 0.25 in identq2 ----
            sq_scr = cpool.tile([P, D], BF16, name=f"sq{r}", tag="sq")
            ssq = stats.tile([P, 1], F32, name=f"ssq{r}", tag="ssq")
            nc.vector.scalar_tensor_tensor(
                out=sq_scr, in0=csum, scalar=1.0, in1=csum,
                op0=mybir.AluOpType.mult, op1=mybir.AluOpType.mult,
                accum_out=ssq)
            lnv = stats.tile([P, 1], F32, name=f"ln{r}", tag="ln")
            nc.scalar.activation(lnv, ssq, AF.Ln, scale=16.0)
            rs = stats.tile([P, 1], F32, name=f"rs{r}", tag="rs")
            nc.scalar.activation(rs, lnv, AF.Exp, scale=-0.5)
            rs_tiles[r] = rs

            # ---- transpose via matmul against the identity ----
            # all 4 transposes share one PSUM bank at different offsets
            cT = ctpool.tile([P, NT, P], FP8, name=f"cT{r}", tag="cT")
            pt = wpsum.tile([P, NT, P], F32, name=f"pt{r}", tag="pt")
            for t in range(NT):
                nc.tensor.matmul(pt[:, t, :], csum[:, t * P:(t + 1) * P],
                                 ident, start=True, stop=True)
            for h in range(2):
                nc.vector.tensor_copy(cT[:, 2 * h:2 * h + 2, :],
                                      pt[:, 2 * h:2 * h + 2, :])

            # ---- main matmuls + exp, two pairs per wave ----
            e_tiles[r] = []
            for w0 in range(2):            # wave: pairs (2*w0, 2*w0+1)
                pss = []
                for pj in (2 * w0, 2 * w0 + 1):
                    ps = mpsum.tile([P, 2, 512], F32, name=f"ps{r}_{pj}",
                                    tag="ps")
                    pss.append(ps)
                for h in range(2):
                    for pi, pj in enumerate((2 * w0, 2 * w0 + 1)):
                        for b in range(2):
                            bite = 2 * pj + b
                            nc.tensor.matmul(
                                pss[pi][:, b, :],
                                cT[:, 2 * h:2 * h + 2, :],
                                ut_sb[:, 2 * h:2 * h + 2,
                                      bite * 512:(bite + 1) * 512],
                                start=(h == 0), stop=(h == 1),
                                perf_mode=DR)
                for pi, pj in enumerate((2 * w0, 2 * w0 + 1)):
                    e = epool.tile([P, 2 * 512], BF16, name=f"e{r}_{pj}",
                                   tag="e")
                    nc.scalar.activation(
                        e, pss[pi].rearrange("p a b -> p (a b)"),
                        AF.Exp, scale=rs[:, 0:1])
                    e_tiles[r].append(e)

            # previous chunk's partition-sum matmuls, after this chunk's
            # main matmuls on the PE FIFO
            if r >= 1:
                _sum_mms(nc, sum_ps, ones, em_tiles[r - 1], r - 1)

        # ---- tail: last chunk's epilogue + outputs ----
        epilogue(NCHUNK - 1)
        nc.sync.dma_start(out=out[8].rearrange("(r p) -> p r", p=P),
                          in_=e_diag)
        _sum_mms(nc, sum_ps, ones, em_tiles[NCHUNK - 1], NCHUNK - 1)
        s_sb = [singles.tile([P, 512], F32, name=f"ssb{j}") for j in range(2)]
        for j in range(2):
            nc.vector.tensor_copy(s_sb[j], sum_ps[j])
        for bite in range(NB):
            j, s = bite // 4, bite % 4
            nc.sync.dma_start(out=out[bite],
                              in_=s_sb[j][32 * s:32 * s + 1, :])


def _sum_mms(nc, sum_ps, ones, em_list, r):
    for pj, em in em_list:
        for b in range(2):
            bite = 2 * pj + b
            j, s = bite // 4, bite % 4
            nc.tensor.matmul(
                sum_ps[j][32 * s:32 * s + 1, :],
                ones,
                em[:, b * 512:(b + 1) * 512],
                start=(r == 0), stop=(r == NCHUNK - 1),
                tile_position=(0, 32 * s))


_CACHE = {}


def _get_program():
    if "nc" not in _CACHE:
        _CACHE["nc"] = build_program()
    return _CACHE["nc"]


def _prep_inputs(dvecs, w_val):
    dv = np.asarray(dvecs, dtype=np.float32)
    x8 = dv.astype(ml_dtypes.float8_e4m3)                   # (N, M, D)
    u = dv[:, M - 1, :].astype(np.float64)                  # (N, D)
    unorm = np.sqrt((u * u).sum(axis=1))                    # (N,)
    scale = (UT_BOOST * w_val) / unorm                      # (N,)
    utw = (u * scale[:, None]).T.astype(np.float32)         # (D, N)
    ut8 = utw.astype(ml_dtypes.float8_e4m3)
    in_maps = []
    for c in range(N_CORES):
        in_maps.append({
            "x": np.ascontiguousarray(x8[c * LOCAL:(c + 1) * LOCAL]),
            "ut": np.ascontiguousarray(np.roll(ut8, -LOCAL * c, axis=1)),
        })
    return in_maps


def kernel(dvecs, w, b):
    w_val = float(np.asarray(w))
    nc = _get_program()
    in_maps = _prep_inputs(dvecs, w_val)
    res = run_bass_kernel_spmd(nc, in_maps, core_ids=list(range(N_CORES)))
    s_tot = np.zeros(N, dtype=np.float64)
    diag_e = np.zeros(N, dtype=np.float64)
    for c in range(N_CORES):
        o = np.asarray(res.results[c]["out"], dtype=np.float64)
        s_tot += np.roll(o[:8].reshape(N), LOCAL * c)
        diag_e[c * LOCAL:(c + 1) * LOCAL] = o[8]
    cos_d = np.log(np.maximum(diag_e, 1e-300)) / w_val
    rows = np.log(s_tot) - w_val * np.maximum(cos_d, EPS)
    return np.float32(rows.mean())
